# revision 17
# baseline (speedup 1.0000x reference)
"""Trainium2 Bass kernel for causal self-attention with 2D RoPE.

Sharding: batch x head-group parallel over 8 NeuronCores.
  core c -> batch b = c // 4, heads h0 = (c % 4) * 3 .. h0+2.
Each core computes q/k/v projections for its 3 heads, 2D RoPE, causal
flash-attention, and a head-packed output projection accumulated in PSUM.
The host sums the 4 partial outputs per batch.

v2 design:
 - exp split between ACT (exact, scalar engine) and DVE (Schraudolph
   bf16 bit-trick; off-diagonal chunks only) to break the ACT exp wall.
 - causal diag mask added into score PSUM by an identity-stationary
   matmul of a -30000 tile (no vector ops, no memsets per tile).
 - denominator from an appended ones-column in V; per-q reciprocal via
   DVE approx-reciprocal on the [1,S] row; broadcast across partitions
   with a ones-row stationary matmul; attention output scaled by mult.
 - output projection: heads packed (h0,h1 | h2+pad) into K=128
   contractions accumulated in PSUM, DMA'd from PSUM straight to DRAM.
 - all matmuls K=128 so the PE stays in a single tile mode (no drains).
"""

import sys

sys.path.insert(0, "/opt/trn_rl_repo")

import numpy as np
from ml_dtypes import bfloat16

import concourse.bacc as bacc
import concourse.bass as bass
import concourse.mybir as mybir
from concourse import tile
from concourse.bass_utils import run_bass_kernel_spmd

BF = mybir.dt.bfloat16
F32 = mybir.dt.float32
F32R = mybir.dt.float32r
I16 = mybir.dt.int16
AF = mybir.ActivationFunctionType
ALU = mybir.AluOpType

P = 128          # partitions
DM = 768         # d_model
HD = 64          # head dim
NHC = 3          # heads per core
NCC = DM // P    # contraction chunks (6)
SQT = 512        # q-block (matmul moving dim)
QKV = 3 * NHC * HD  # 576

# Schraudolph bf16 exp: bf16_bits = round(x * 128/ln2 + (16256 - C))
SCH_S = 128.0 / float(np.log(2.0))
SCH_C = 6.5
DVE_EXP_MOD = 2  # full chunk ki -> DVE when ki % MOD != 0


def build_program(S=2048, n_devices=8):
    NS = S // P      # seq chunks of 128
    NQ = S // SQT    # q blocks of 512
    KPQ = SQT // P   # k-chunks per q-block (4)

    nc = bacc.Bacc(
        "TRN2", target_bir_lowering=False, debug=False, num_devices=n_devices
    )
    XB = 512
    NXB = S // XB
    xt_d = nc.dram_tensor("xt", [NXB, P, NCC, XB], BF, kind="ExternalInput")
    wqkv_d = nc.dram_tensor("wqkv", [P, NCC, QKV], BF, kind="ExternalInput")
    wo01_d = nc.dram_tensor("wo01", [P, DM], BF, kind="ExternalInput")
    wo2_d = nc.dram_tensor("wo2", [P, DM], BF, kind="ExternalInput")
    cos_d = nc.dram_tensor("cos", [P, NS, HD], BF, kind="ExternalInput")
    sin_d = nc.dram_tensor("sin", [P, NS, HD], BF, kind="ExternalInput")
    trimask_d = nc.dram_tensor("trimask", [P, P], BF, kind="ExternalInput")
    id_d = nc.dram_tensor("ident", [P, P], BF, kind="ExternalInput")
    out_d = nc.dram_tensor("outp", [S, DM], F32, kind="ExternalOutput")
    denr_dram = nc.dram_tensor("den_scratch", [2, SQT], F32, kind="Internal")

    with tile.TileContext(nc) as tc:
        with (
            tc.tile_pool(name="const", bufs=1) as const,
            tc.tile_pool(name="resid", bufs=1) as resid,
        ):
            qk_sb = resid.tile([P, NS, 384], BF)   # rope out: q 0:192 | k 192:384
            tz = resid.tile([P, 6, S], BF)         # j: 0..2 q heads, 3..5 k heads
            v_sb = resid.tile([P, NS, NHC, P], BF)
            ao01 = resid.tile([P, S], BF)
            ao2 = resid.tile([P, S], BF)
            aot = resid.tile([P, SQT], BF)         # h1 staging
            den_sb = resid.tile([P, 2, SQT], F32)  # row 0 live
            denr_sb = resid.tile([P, 2, SQT], F32)  # row 0 live
            wo01_sb = const.tile([P, DM], BF)
            wo2_sb = const.tile([P, DM], BF)
            cos_sb = const.tile([P, NS, HD], BF)
            sin_sb = const.tile([P, NS, HD], BF)
            trimask_sb = const.tile([P, P], BF)
            id_sb = const.tile([P, P], BF)

            # one-time pad zeroing, spread across engines (runs under the
            # initial input DMAs)
            nc.vector.memset(tz[HD:P, 0:3, :], 0.0)           # q transp pads
            nc.vector.memset(ao2[HD:P, :], 0.0)               # outproj K pad
            nc.gpsimd.memset(v_sb[:, :, :, HD], 1.0)          # denominator ones
            nc.gpsimd.memset(v_sb[:, :, :, HD + 1 : P], 0.0)  # v pad cols
            nc.gpsimd.memset(tz[HD:P, 3:6, :], 0.0)           # k transp pads

            wqkv_sb = const.tile([P, NCC, QKV], BF)
            nc.sync.dma_start(wqkv_sb[:, 0:3, :], wqkv_d[:, 0:3, :])
            nc.scalar.dma_start(wqkv_sb[:, 3:NCC, :], wqkv_d[:, 3:NCC, :])
            xt_sb = const.tile([P, NCC, S], BF)
            for b in range(NXB):
                bsl = slice(b * XB, (b + 1) * XB)
                csl = slice(b * (NS // NXB), (b + 1) * (NS // NXB))
                nc.sync.dma_start(cos_sb[:, csl, :], cos_d[:, csl, :])
                nc.scalar.dma_start(sin_sb[:, csl, :], sin_d[:, csl, :])
                nc.sync.dma_start(xt_sb[:, 0:3, bsl], xt_d[b][:, 0:3, :])
                nc.scalar.dma_start(xt_sb[:, 3:NCC, bsl], xt_d[b][:, 3:NCC, :])
            nc.sync.dma_start(id_sb[:], id_d[:])
            nc.scalar.dma_start(trimask_sb[:], trimask_d[:])
            nc.scalar.dma_start(wo01_sb[:], wo01_d[:])
            nc.sync.dma_start(wo2_sb[:], wo2_d[:])

            # ---- phase 1: qkv projection + rope + v pack + transposes ----
            with (
                tc.tile_pool(name="p1ps", bufs=3, space="PSUM") as pp,
                tc.tile_pool(name="p2ps", bufs=2, space="PSUM") as p2,
                tc.tile_pool(name="p1t", bufs=3) as tp,
            ):

                def emit_transposes(s):
                    pt = p2.tile([P, 384], BF, tag="pt", name="pt")
                    for j in range(3):
                        nc.tensor.transpose(
                            pt[:, j * P : (j + 1) * P],
                            qk_sb[:, s, j * P : (j + 1) * P],
                            id_sb[:],
                        )
                    # lo partitions of pt chunks -> tz slots 0,2,4
                    # hi partitions -> slots 1,3,5 (see head/slot mapping)
                    pstep = tz.ap[0][0]
                    lo = bass.AP(
                        tz.tensor, tz.offset + s * P,
                        [[pstep, HD], [2 * S, 3], [1, P]],
                    )
                    hi = bass.AP(
                        tz.tensor, tz.offset + S + s * P,
                        [[pstep, HD], [2 * S, 3], [1, P]],
                    )
                    src3 = pt.rearrange("p (a b) -> p a b", b=P)
                    nc.scalar.copy(lo, src3[0:HD, :, :])
                    nc.vector.tensor_copy(hi, src3[HD:P, :, :])

                for s in range(NS):
                    pqkv = pp.tile([P, QKV], F32, tag="pqkv", name="pqkv")
                    xsl = xt_sb[:, :, s * P : (s + 1) * P]
                    for c in range(NCC):
                        st, sp = (c == 0), (c == NCC - 1)
                        nc.tensor.matmul(
                            pqkv[:, 0:512], xsl[:, c, :], wqkv_sb[:, c, 0:512],
                            start=st, stop=sp,
                        )
                        nc.tensor.matmul(
                            pqkv[:, 512:QKV], xsl[:, c, :], wqkv_sb[:, c, 512:QKV],
                            start=st, stop=sp,
                        )
                    if s >= 1:
                        emit_transposes(s - 1)
                    qk = pqkv[:, 0:384]
                    qk3 = qk.rearrange("p (b x) -> p b x", x=32)
                    cs = cos_sb[:, s, :]
                    sn = sin_sb[:, s, :]
                    c3b = bass.AP(cs.tensor, cs.offset, [cs.ap[0], [0, 6], [1, HD]])
                    s3a = bass.AP(sn.tensor, sn.offset, [sn.ap[0], [0, 6], [32, 2], [1, 16]])
                    s3b = bass.AP(sn.tensor, sn.offset + 16, [sn.ap[0], [0, 6], [32, 2], [1, 16]])
                    t = tp.tile([P, 384], F32, tag="ropet", name="t")
                    t3 = t.rearrange("p (b x) -> p b x", x=32)
                    nc.vector.tensor_tensor(
                        t3[:, :, 0:16], qk3[:, :, 16:32], s3a, ALU.mult
                    )
                    nc.vector.tensor_tensor(
                        t3[:, :, 16:32], qk3[:, :, 0:16], s3b, ALU.mult
                    )
                    t2 = tp.tile([P, 384], F32, tag="ropet2", name="t2")
                    nc.vector.tensor_tensor(t2[:], qk[:], c3b, ALU.mult)
                    nc.gpsimd.tensor_tensor(
                        qk_sb[:, s, 0:192], t2[:, 0:192], t[:, 0:192], ALU.add
                    )
                    nc.gpsimd.tensor_tensor(
                        qk_sb[:, s, 192:384], t2[:, 192:384], t[:, 192:384], ALU.add
                    )
                    nc.scalar.copy(
                        v_sb[:, s, :, 0:HD],
                        pqkv[:, 384:QKV].rearrange("p (h x) -> p h x", x=HD),
                    )
                emit_transposes(NS - 1)

            # ---- phase 2: attention + den + outproj ----
            with (
                tc.tile_pool(name="scps", bufs=3, space="PSUM") as scp,
                tc.tile_pool(name="aops", bufs=2, space="PSUM") as aop,
                tc.tile_pool(name="expp", bufs=10) as expp,
                tc.tile_pool(name="outb", bufs=3) as outb,
                tc.tile_pool(name="denbp", bufs=2) as denbp,
            ):
                den_jobs = []  # (h, qj, pa) pending den-broadcast + scale

                def flush_den(keep):
                    while len(den_jobs) > keep:
                        h, qj, pa = den_jobs.pop(0)
                        slot = (NHC * qj + h) % 2
                        qsl = slice(qj * SQT, (qj + 1) * SQT)
                        denb = denbp.tile([P, SQT], F32, tag="denb", name="denb")
                        dr = denr_dram[slot, :]
                        bsrc = bass.AP(
                            dr.tensor, dr.offset, [[0, P], [1, SQT]]
                        )
                        nc.gpsimd.dma_start(denb[:], bsrc)
                        if h == 0:
                            dst = ao01[0:HD, qsl]
                        elif h == 1:
                            dst = aot[0:HD, :]
                        else:
                            dst = ao2[0:HD, qsl]
                        nc.vector.tensor_tensor(
                            dst, pa[0:HD, :], denb[0:HD, :], ALU.mult
                        )
                        if h == 1:
                            nc.scalar.copy(ao01[HD:P, qsl], aot[0:HD, :])

                def attention_head(qj, h):
                    nki = KPQ * qj + KPQ
                    flush_den(1)
                    pa = aop.tile([P, SQT], F32, tag="pa", name="pa")
                    av_q = []  # (ki, off, e, j)

                    def emit_av(lag):
                        while av_q and len(av_q) > lag:
                            ki, off, e, j = av_q.pop(0)
                            nc.tensor.matmul(
                                pa[:, off:SQT],
                                v_sb[:, ki, h, :],
                                e[:, j, off:SQT],
                                start=(ki == 0), stop=(ki == nki - 1),
                                skip_group_check=True,
                            )

                    for pi in range(nki // 2):
                        k0 = 2 * pi
                        diag = (k0 - KPQ * qj) >= -1
                        pp = scp.tile([P, 2, SQT], F32, tag="ps", name="ps")
                        offs = []
                        for j, ki in enumerate((k0, k0 + 1)):
                            r = ki - KPQ * qj
                            off = max(r, 0) * P
                            offs.append(off)
                            nc.tensor.matmul(
                                pp[:, j, off:SQT],
                                tz[:, 3 + h, ki * P : (ki + 1) * P],
                                tz[:, h, qj * SQT + off : (qj + 1) * SQT],
                                start=True, stop=(r < 0),
                                skip_group_check=True,
                            )
                            if r >= 0:
                                nc.tensor.matmul(
                                    pp[:, j, off : off + P],
                                    id_sb[:],
                                    trimask_sb[:],
                                    start=False, stop=True,
                                    skip_group_check=True,
                                )
                        e = expp.tile([P, 2, SQT], BF, tag="e", name="e")
                        if diag:
                            for j, ki in enumerate((k0, k0 + 1)):
                                nc.scalar.activation(
                                    e[:, j, offs[j] : SQT],
                                    pp[:, j, offs[j] : SQT],
                                    AF.Exp, scale=0.125,
                                )
                        elif (pi % 3) != 0:
                            nc.vector.tensor_scalar(
                                e.bitcast(I16)[:, :, :],
                                pp[:, :, :],
                                SCH_S * 0.125,
                                16256.0 - SCH_C,
                                ALU.mult,
                                ALU.add,
                            )
                        else:
                            nc.scalar.activation(
                                e[:, :, :], pp[:, :, :], AF.Exp, scale=0.125
                            )
                        av_q.append((k0, offs[0], e, 0))
                        av_q.append((k0 + 1, offs[1], e, 1))
                        emit_av(4)
                    emit_av(0)
                    slot = (NHC * qj + h) % 2
                    deng = nc.vector.tensor_copy if slot else nc.scalar.copy
                    deng(den_sb[0:1, slot, :], pa[HD : HD + 1, :])
                    nc.vector.reciprocal_approx_fast(
                        denr_sb[0:1, slot, :], den_sb[0:1, slot, :]
                    )
                    nc.gpsimd.dma_start(
                        denr_dram[slot, :], denr_sb[0:1, slot, :]
                    )
                    den_jobs.append((h, qj, pa))

                def outproj_block(qj):
                    for s in range(qj * KPQ, (qj + 1) * KPQ):
                        sl = slice(s * P, (s + 1) * P)
                        po = scp.tile([P, 2, SQT], F32, tag="ps", name="po")
                        pof = po.rearrange("p a b -> p (a b)")
                        for lo, hi in ((0, 512), (512, DM)):
                            nc.tensor.matmul(
                                pof[:, lo:hi], ao01[:, sl], wo01_sb[:, lo:hi],
                                start=True, stop=False,
                            )
                            nc.tensor.matmul(
                                pof[:, lo:hi], ao2[:, sl], wo2_sb[:, lo:hi],
                                start=False, stop=True,
                            )
                        ob = outb.tile([P, DM], F32, tag="ob", name="ob")
                        nc.scalar.copy(ob[:, 0:512], pof[:, 0:512])
                        nc.vector.tensor_copy(ob[:, 512:DM], pof[:, 512:DM])
                        nc.sync.dma_start(out_d[sl, :], ob[:])

                order = list(range(NQ - 1, -1, -1))
                for idx, qj in enumerate(order):
                    for h in range(NHC):
                        attention_head(qj, h)
                        if idx >= 1 and h == 1:
                            outproj_block(order[idx - 1])
                flush_den(0)
                outproj_block(order[-1])

    nc.compile()
    return nc


_cache = {}
LAST_RESULT = None


def _get_program(S, n_devices):
    key = (S, n_devices)
    if key not in _cache:
        _cache[key] = build_program(S, n_devices)
    return _cache[key]


def _rope_tables(row_ids, col_ids, S):
    inv = 1.0 / (10000.0 ** (np.arange(0, 32, 2, dtype=np.float64) / 32.0))

    def block(ids):
        ang = ids.astype(np.float64)[:, None] * inv[None, :]
        c = np.concatenate([np.cos(ang), np.cos(ang)], -1)
        s_ = np.concatenate([-np.sin(ang), np.sin(ang)], -1)  # signed (shuffle form)
        return c, s_

    cr, sr = block(np.asarray(row_ids))
    cc, sc = block(np.asarray(col_ids))
    cos64 = np.concatenate([cr, cc], -1)
    sin64 = np.concatenate([sr, sc], -1)
    return cos64.astype(bfloat16), sin64.astype(bfloat16)


def kernel(x, row_ids, col_ids, Wq, Wk, Wv, Wo):
    x = np.asarray(x)
    B, S, _ = x.shape
    n_cores = 8
    groups = n_cores // B  # head groups per batch (4)
    hpg = NHC

    nc = _get_program(S, n_cores)
    cos_t, sin_t = _rope_tables(row_ids, col_ids, S)
    cos_t = np.ascontiguousarray(cos_t.reshape(S // P, P, -1).transpose(1, 0, 2))
    sin_t = np.ascontiguousarray(sin_t.reshape(S // P, P, -1).transpose(1, 0, 2))

    pp_ = np.arange(P)[:, None]
    ff = np.arange(P)[None, :]
    trimask = np.where(pp_ > ff, -30000.0, 0.0).astype(np.float32).astype(bfloat16)
    ident = np.eye(P, dtype=bfloat16)

    Wq, Wk, Wv, Wo = (np.asarray(w, np.float32) for w in (Wq, Wk, Wv, Wo))
    in_maps = []
    for c in range(n_cores):
        b = c // groups
        h0 = (c % groups) * hpg
        rows = slice(h0 * HD, (h0 + hpg) * HD)
        xt = np.ascontiguousarray(x[b].T).astype(bfloat16)
        NXB = S // 512
        xt = np.ascontiguousarray(
            xt.reshape(NCC, P, NXB, 512).transpose(2, 1, 0, 3)
        )
        wqkv = np.concatenate(
            [Wq[rows].T, Wk[rows].T, Wv[rows].T], axis=1
        ).astype(bfloat16)
        wqkv = np.ascontiguousarray(wqkv.reshape(NCC, P, QKV).transpose(1, 0, 2))
        wo = np.ascontiguousarray(Wo[:, rows].T)  # [192, 768]
        wo01 = np.ascontiguousarray(wo[0:P, :]).astype(bfloat16)
        wo2 = np.zeros((P, DM), np.float32)
        wo2[0:HD, :] = wo[P : P + HD, :]
        wo2 = wo2.astype(bfloat16)
        in_maps.append(
            {
                "xt": xt,
                "wqkv": wqkv,
                "wo01": wo01,
                "wo2": wo2,
                "cos": cos_t,
                "sin": sin_t,
                "trimask": trimask,
                "ident": ident,
            }
        )

    import os

    trace = bool(os.environ.get("KERNEL_TRACE"))
    kw = {}
    if trace and os.environ.get("KERNEL_TRACE_DIR"):
        kw["tmpdir"] = os.environ["KERNEL_TRACE_DIR"]
    res = run_bass_kernel_spmd(nc, in_maps, list(range(n_cores)), trace=trace, **kw)
    global LAST_RESULT
    LAST_RESULT = res

    outs = [res.results[c]["outp"] for c in range(n_cores)]
    out = np.stack(
        [sum(outs[b * groups + g] for g in range(groups)) for b in range(B)], axis=0
    )
    return out.astype(np.float32)


# revision 18
# speedup vs baseline: 1.0009x; 1.0009x over previous
"""Trainium2 Bass kernel for causal self-attention with 2D RoPE.

Sharding: batch x head-group parallel over 8 NeuronCores.
  core c -> batch b = c // 4, heads h0 = (c % 4) * 3 .. h0+2.
Each core computes q/k/v projections for its 3 heads, 2D RoPE, causal
flash-attention, and a head-packed output projection accumulated in PSUM.
The host sums the 4 partial outputs per batch.

v2 design:
 - exp split between ACT (exact, scalar engine) and DVE (Schraudolph
   bf16 bit-trick; off-diagonal chunks only) to break the ACT exp wall.
 - causal diag mask added into score PSUM by an identity-stationary
   matmul of a -30000 tile (no vector ops, no memsets per tile).
 - denominator from an appended ones-column in V; per-q reciprocal via
   DVE approx-reciprocal on the [1,S] row; broadcast across partitions
   with a ones-row stationary matmul; attention output scaled by mult.
 - output projection: heads packed (h0,h1 | h2+pad) into K=128
   contractions accumulated in PSUM, DMA'd from PSUM straight to DRAM.
 - all matmuls K=128 so the PE stays in a single tile mode (no drains).
"""

import sys

sys.path.insert(0, "/opt/trn_rl_repo")

import numpy as np
from ml_dtypes import bfloat16

import concourse.bacc as bacc
import concourse.bass as bass
import concourse.mybir as mybir
from concourse import tile
from concourse.bass_utils import run_bass_kernel_spmd

BF = mybir.dt.bfloat16
F32 = mybir.dt.float32
F32R = mybir.dt.float32r
I16 = mybir.dt.int16
AF = mybir.ActivationFunctionType
ALU = mybir.AluOpType

P = 128          # partitions
DM = 768         # d_model
HD = 64          # head dim
NHC = 3          # heads per core
NCC = DM // P    # contraction chunks (6)
SQT = 512        # q-block (matmul moving dim)
QKV = 3 * NHC * HD  # 576

# Schraudolph bf16 exp: bf16_bits = round(x * 128/ln2 + (16256 - C))
SCH_S = 128.0 / float(np.log(2.0))
SCH_C = 6.5
DVE_EXP_MOD = 2  # full chunk ki -> DVE when ki % MOD != 0


def build_program(S=2048, n_devices=8):
    NS = S // P      # seq chunks of 128
    NQ = S // SQT    # q blocks of 512
    KPQ = SQT // P   # k-chunks per q-block (4)

    nc = bacc.Bacc(
        "TRN2", target_bir_lowering=False, debug=False, num_devices=n_devices
    )
    XB = 512
    NXB = S // XB
    xt_d = nc.dram_tensor("xt", [NXB, P, NCC, XB], BF, kind="ExternalInput")
    wqkv_d = nc.dram_tensor("wqkv", [P, NCC, QKV], BF, kind="ExternalInput")
    wo01_d = nc.dram_tensor("wo01", [P, DM], BF, kind="ExternalInput")
    wo2_d = nc.dram_tensor("wo2", [P, DM], BF, kind="ExternalInput")
    cos_d = nc.dram_tensor("cos", [P, NS, HD], BF, kind="ExternalInput")
    sin_d = nc.dram_tensor("sin", [P, NS, HD], BF, kind="ExternalInput")
    trimask_d = nc.dram_tensor("trimask", [P, P], BF, kind="ExternalInput")
    id_d = nc.dram_tensor("ident", [P, P], BF, kind="ExternalInput")
    out_d = nc.dram_tensor("outp", [S, DM], F32, kind="ExternalOutput")
    denr_dram = nc.dram_tensor("den_scratch", [2, SQT], F32, kind="Internal")

    with tile.TileContext(nc) as tc:
        with (
            tc.tile_pool(name="const", bufs=1) as const,
            tc.tile_pool(name="resid", bufs=1) as resid,
        ):
            qk_sb = resid.tile([P, NS, 384], BF)   # rope out: q 0:192 | k 192:384
            tz = resid.tile([P, 6, S], BF)         # j: 0..2 q heads, 3..5 k heads
            v_sb = resid.tile([P, NS, NHC, P], BF)
            ao01 = resid.tile([P, S], BF)
            ao2 = resid.tile([P, S], BF)
            aot = resid.tile([P, SQT], BF)         # h1 staging
            den_sb = resid.tile([P, 2, SQT], F32)  # row 0 live
            denr_sb = resid.tile([P, 2, SQT], F32)  # row 0 live
            wo01_sb = const.tile([P, DM], BF)
            wo2_sb = const.tile([P, DM], BF)
            cos_sb = const.tile([P, NS, HD], BF)
            sin_sb = const.tile([P, NS, HD], BF)
            trimask_sb = const.tile([P, P], BF)
            id_sb = const.tile([P, P], BF)

            # one-time pad zeroing, spread across engines (runs under the
            # initial input DMAs)
            nc.vector.memset(tz[HD:P, 0:3, :], 0.0)           # q transp pads
            nc.vector.memset(ao2[HD:P, :], 0.0)               # outproj K pad
            nc.gpsimd.memset(v_sb[:, :, :, HD], 1.0)          # denominator ones
            nc.gpsimd.memset(v_sb[:, :, :, HD + 1 : P], 0.0)  # v pad cols
            nc.gpsimd.memset(tz[HD:P, 3:6, :], 0.0)           # k transp pads

            wqkv_sb = const.tile([P, NCC, QKV], BF)
            nc.sync.dma_start(wqkv_sb[:, 0:3, :], wqkv_d[:, 0:3, :])
            nc.scalar.dma_start(wqkv_sb[:, 3:NCC, :], wqkv_d[:, 3:NCC, :])
            xt_sb = const.tile([P, NCC, S], BF)
            for b in range(NXB):
                bsl = slice(b * XB, (b + 1) * XB)
                csl = slice(b * (NS // NXB), (b + 1) * (NS // NXB))
                nc.sync.dma_start(cos_sb[:, csl, :], cos_d[:, csl, :])
                nc.scalar.dma_start(sin_sb[:, csl, :], sin_d[:, csl, :])
                nc.sync.dma_start(xt_sb[:, 0:3, bsl], xt_d[b][:, 0:3, :])
                nc.scalar.dma_start(xt_sb[:, 3:NCC, bsl], xt_d[b][:, 3:NCC, :])
            nc.sync.dma_start(id_sb[:], id_d[:])
            nc.scalar.dma_start(trimask_sb[:], trimask_d[:])
            nc.scalar.dma_start(wo01_sb[:], wo01_d[:])
            nc.sync.dma_start(wo2_sb[:], wo2_d[:])

            # ---- phase 1: qkv projection + rope + v pack + transposes ----
            with (
                tc.tile_pool(name="p1ps", bufs=3, space="PSUM") as pp,
                tc.tile_pool(name="p2ps", bufs=2, space="PSUM") as p2,
                tc.tile_pool(name="p1t", bufs=3) as tp,
            ):

                def emit_transposes(s):
                    pt = p2.tile([P, 384], BF, tag="pt", name="pt")
                    for j in range(3):
                        nc.tensor.transpose(
                            pt[:, j * P : (j + 1) * P],
                            qk_sb[:, s, j * P : (j + 1) * P],
                            id_sb[:],
                        )
                    # lo partitions of pt chunks -> tz slots 0,2,4
                    # hi partitions -> slots 1,3,5 (see head/slot mapping)
                    pstep = tz.ap[0][0]
                    lo = bass.AP(
                        tz.tensor, tz.offset + s * P,
                        [[pstep, HD], [2 * S, 3], [1, P]],
                    )
                    hi = bass.AP(
                        tz.tensor, tz.offset + S + s * P,
                        [[pstep, HD], [2 * S, 3], [1, P]],
                    )
                    src3 = pt.rearrange("p (a b) -> p a b", b=P)
                    nc.scalar.copy(lo, src3[0:HD, :, :])
                    nc.vector.tensor_copy(hi, src3[HD:P, :, :])

                for s in range(NS):
                    pqkv = pp.tile([P, QKV], F32, tag="pqkv", name="pqkv")
                    xsl = xt_sb[:, :, s * P : (s + 1) * P]
                    for c in range(NCC):
                        st, sp = (c == 0), (c == NCC - 1)
                        nc.tensor.matmul(
                            pqkv[:, 0:512], xsl[:, c, :], wqkv_sb[:, c, 0:512],
                            start=st, stop=sp,
                        )
                        nc.tensor.matmul(
                            pqkv[:, 512:QKV], xsl[:, c, :], wqkv_sb[:, c, 512:QKV],
                            start=st, stop=sp,
                        )
                    if s >= 1:
                        emit_transposes(s - 1)
                    qk = pqkv[:, 0:384]
                    qk3 = qk.rearrange("p (b x) -> p b x", x=32)
                    cs = cos_sb[:, s, :]
                    sn = sin_sb[:, s, :]
                    c3b = bass.AP(cs.tensor, cs.offset, [cs.ap[0], [0, 6], [1, HD]])
                    s3a = bass.AP(sn.tensor, sn.offset, [sn.ap[0], [0, 6], [32, 2], [1, 16]])
                    s3b = bass.AP(sn.tensor, sn.offset + 16, [sn.ap[0], [0, 6], [32, 2], [1, 16]])
                    t = tp.tile([P, 384], F32, tag="ropet", name="t")
                    t3 = t.rearrange("p (b x) -> p b x", x=32)
                    nc.vector.tensor_tensor(
                        t3[:, :, 0:16], qk3[:, :, 16:32], s3a, ALU.mult
                    )
                    nc.vector.tensor_tensor(
                        t3[:, :, 16:32], qk3[:, :, 0:16], s3b, ALU.mult
                    )
                    t2 = tp.tile([P, 384], F32, tag="ropet2", name="t2")
                    nc.vector.tensor_tensor(t2[:], qk[:], c3b, ALU.mult)
                    nc.gpsimd.tensor_tensor(
                        qk_sb[:, s, 0:192], t2[:, 0:192], t[:, 0:192], ALU.add
                    )
                    nc.gpsimd.tensor_tensor(
                        qk_sb[:, s, 192:384], t2[:, 192:384], t[:, 192:384], ALU.add
                    )
                    nc.scalar.copy(
                        v_sb[:, s, :, 0:HD],
                        pqkv[:, 384:QKV].rearrange("p (h x) -> p h x", x=HD),
                    )
                emit_transposes(NS - 1)

            # ---- phase 2: attention + den + outproj ----
            with (
                tc.tile_pool(name="scps", bufs=6, space="PSUM") as scp,
                tc.tile_pool(name="aops", bufs=2, space="PSUM") as aop,
                tc.tile_pool(name="expp", bufs=10) as expp,
                tc.tile_pool(name="outb", bufs=3) as outb,
                tc.tile_pool(name="denbp", bufs=2) as denbp,
            ):
                den_jobs = []  # (h, qj, pa) pending den-broadcast + scale

                def flush_den(keep):
                    while len(den_jobs) > keep:
                        h, qj, pa = den_jobs.pop(0)
                        slot = (NHC * qj + h) % 2
                        qsl = slice(qj * SQT, (qj + 1) * SQT)
                        denb = denbp.tile([P, SQT], F32, tag="denb", name="denb")
                        dr = denr_dram[slot, :]
                        bsrc = bass.AP(
                            dr.tensor, dr.offset, [[0, P], [1, SQT]]
                        )
                        nc.gpsimd.dma_start(denb[:], bsrc)
                        if h == 0:
                            dst = ao01[0:HD, qsl]
                        elif h == 1:
                            dst = aot[0:HD, :]
                        else:
                            dst = ao2[0:HD, qsl]
                        nc.vector.tensor_tensor(
                            dst, pa[0:HD, :], denb[0:HD, :], ALU.mult
                        )
                        if h == 1:
                            nc.vector.tensor_copy(ao01[HD:P, qsl], aot[0:HD, :])

                def attention_head(qj, h):
                    nki = KPQ * qj + KPQ
                    flush_den(1)
                    pa = aop.tile([P, SQT], F32, tag="pa", name="pa")
                    av_q = []  # (ki, off, e, j)

                    def emit_av(lag):
                        while av_q and len(av_q) > lag:
                            ki, off, e, j = av_q.pop(0)
                            nc.tensor.matmul(
                                pa[:, off:SQT],
                                v_sb[:, ki, h, :],
                                e[:, off:SQT],
                                start=(ki == 0), stop=(ki == nki - 1),
                                skip_group_check=True,
                            )

                    for ki in range(nki):
                        r = ki - KPQ * qj
                        off = max(r, 0) * P
                        ps = scp.tile([P, SQT], F32, tag="ps", name="ps")
                        nc.tensor.matmul(
                            ps[:, off:SQT],
                            tz[:, 3 + h, ki * P : (ki + 1) * P],
                            tz[:, h, qj * SQT + off : (qj + 1) * SQT],
                            start=True, stop=(r < 0),
                            skip_group_check=True,
                        )
                        if r >= 0:
                            nc.tensor.matmul(
                                ps[:, off : off + P],
                                id_sb[:],
                                trimask_sb[:],
                                start=False, stop=True,
                                skip_group_check=True,
                            )
                        e = expp.tile([P, SQT], BF, tag="e", name="e")
                        if r < 0 and (ki % 3) != 0:
                            nc.vector.tensor_scalar(
                                e.bitcast(I16)[:, off:SQT],
                                ps[:, off:SQT],
                                SCH_S * 0.125,
                                16256.0 - SCH_C,
                                ALU.mult,
                                ALU.add,
                            )
                        else:
                            nc.scalar.activation(
                                e[:, off:SQT], ps[:, off:SQT], AF.Exp, scale=0.125
                            )
                        av_q.append((ki, off, e, 0))
                        emit_av(4)
                    emit_av(0)
                    slot = (NHC * qj + h) % 2
                    deng = nc.vector.tensor_copy if slot else nc.scalar.copy
                    deng(den_sb[0:1, slot, :], pa[HD : HD + 1, :])
                    nc.vector.reciprocal_approx_fast(
                        denr_sb[0:1, slot, :], den_sb[0:1, slot, :]
                    )
                    nc.gpsimd.dma_start(
                        denr_dram[slot, :], denr_sb[0:1, slot, :]
                    )
                    den_jobs.append((h, qj, pa))

                def outproj_block(qj):
                    for s in range(qj * KPQ, (qj + 1) * KPQ):
                        sl = slice(s * P, (s + 1) * P)
                        poa = scp.tile([P, SQT], F32, tag="ps", name="poa")
                        pob = scp.tile([P, SQT], F32, tag="ps", name="pob")
                        for po, lo, hi in ((poa, 0, 512), (pob, 512, DM)):
                            w = hi - lo
                            nc.tensor.matmul(
                                po[:, 0:w], ao01[:, sl], wo01_sb[:, lo:hi],
                                start=True, stop=False,
                            )
                            nc.tensor.matmul(
                                po[:, 0:w], ao2[:, sl], wo2_sb[:, lo:hi],
                                start=False, stop=True,
                            )
                        ob = outb.tile([P, DM], F32, tag="ob", name="ob")
                        nc.scalar.copy(ob[:, 0:512], poa[:, 0:512])
                        nc.vector.tensor_copy(ob[:, 512:DM], pob[:, 0:256])
                        nc.sync.dma_start(out_d[sl, :], ob[:])

                order = list(range(NQ - 1, -1, -1))
                for idx, qj in enumerate(order):
                    for h in range(NHC):
                        attention_head(qj, h)
                        if idx >= 1 and h == 1:
                            outproj_block(order[idx - 1])
                flush_den(0)
                outproj_block(order[-1])

    nc.compile()
    return nc


_cache = {}
LAST_RESULT = None


def _get_program(S, n_devices):
    key = (S, n_devices)
    if key not in _cache:
        _cache[key] = build_program(S, n_devices)
    return _cache[key]


def _rope_tables(row_ids, col_ids, S):
    inv = 1.0 / (10000.0 ** (np.arange(0, 32, 2, dtype=np.float64) / 32.0))

    def block(ids):
        ang = ids.astype(np.float64)[:, None] * inv[None, :]
        c = np.concatenate([np.cos(ang), np.cos(ang)], -1)
        s_ = np.concatenate([-np.sin(ang), np.sin(ang)], -1)  # signed (shuffle form)
        return c, s_

    cr, sr = block(np.asarray(row_ids))
    cc, sc = block(np.asarray(col_ids))
    cos64 = np.concatenate([cr, cc], -1)
    sin64 = np.concatenate([sr, sc], -1)
    return cos64.astype(bfloat16), sin64.astype(bfloat16)


def kernel(x, row_ids, col_ids, Wq, Wk, Wv, Wo):
    x = np.asarray(x)
    B, S, _ = x.shape
    n_cores = 8
    groups = n_cores // B  # head groups per batch (4)
    hpg = NHC

    nc = _get_program(S, n_cores)
    cos_t, sin_t = _rope_tables(row_ids, col_ids, S)
    cos_t = np.ascontiguousarray(cos_t.reshape(S // P, P, -1).transpose(1, 0, 2))
    sin_t = np.ascontiguousarray(sin_t.reshape(S // P, P, -1).transpose(1, 0, 2))

    pp_ = np.arange(P)[:, None]
    ff = np.arange(P)[None, :]
    trimask = np.where(pp_ > ff, -30000.0, 0.0).astype(np.float32).astype(bfloat16)
    ident = np.eye(P, dtype=bfloat16)

    Wq, Wk, Wv, Wo = (np.asarray(w, np.float32) for w in (Wq, Wk, Wv, Wo))
    in_maps = []
    for c in range(n_cores):
        b = c // groups
        h0 = (c % groups) * hpg
        rows = slice(h0 * HD, (h0 + hpg) * HD)
        xt = np.ascontiguousarray(x[b].T).astype(bfloat16)
        NXB = S // 512
        xt = np.ascontiguousarray(
            xt.reshape(NCC, P, NXB, 512).transpose(2, 1, 0, 3)
        )
        wqkv = np.concatenate(
            [Wq[rows].T, Wk[rows].T, Wv[rows].T], axis=1
        ).astype(bfloat16)
        wqkv = np.ascontiguousarray(wqkv.reshape(NCC, P, QKV).transpose(1, 0, 2))
        wo = np.ascontiguousarray(Wo[:, rows].T)  # [192, 768]
        wo01 = np.ascontiguousarray(wo[0:P, :]).astype(bfloat16)
        wo2 = np.zeros((P, DM), np.float32)
        wo2[0:HD, :] = wo[P : P + HD, :]
        wo2 = wo2.astype(bfloat16)
        in_maps.append(
            {
                "xt": xt,
                "wqkv": wqkv,
                "wo01": wo01,
                "wo2": wo2,
                "cos": cos_t,
                "sin": sin_t,
                "trimask": trimask,
                "ident": ident,
            }
        )

    import os

    trace = bool(os.environ.get("KERNEL_TRACE"))
    kw = {}
    if trace and os.environ.get("KERNEL_TRACE_DIR"):
        kw["tmpdir"] = os.environ["KERNEL_TRACE_DIR"]
    res = run_bass_kernel_spmd(nc, in_maps, list(range(n_cores)), trace=trace, **kw)
    global LAST_RESULT
    LAST_RESULT = res

    outs = [res.results[c]["outp"] for c in range(n_cores)]
    out = np.stack(
        [sum(outs[b * groups + g] for g in range(groups)) for b in range(B)], axis=0
    )
    return out.astype(np.float32)


# revision 19
# speedup vs baseline: 1.0061x; 1.0052x over previous
"""Trainium2 Bass kernel for causal self-attention with 2D RoPE.

Sharding: batch x head-group parallel over 8 NeuronCores.
  core c -> batch b = c // 4, heads h0 = (c % 4) * 3 .. h0+2.
Each core computes q/k/v projections for its 3 heads, 2D RoPE, causal
flash-attention, and a head-packed output projection accumulated in PSUM.
The host sums the 4 partial outputs per batch.

v2 design:
 - exp split between ACT (exact, scalar engine) and DVE (Schraudolph
   bf16 bit-trick; off-diagonal chunks only) to break the ACT exp wall.
 - causal diag mask added into score PSUM by an identity-stationary
   matmul of a -30000 tile (no vector ops, no memsets per tile).
 - denominator from an appended ones-column in V; per-q reciprocal via
   DVE approx-reciprocal on the [1,S] row; broadcast across partitions
   with a ones-row stationary matmul; attention output scaled by mult.
 - output projection: heads packed (h0,h1 | h2+pad) into K=128
   contractions accumulated in PSUM, DMA'd from PSUM straight to DRAM.
 - all matmuls K=128 so the PE stays in a single tile mode (no drains).
"""

import sys

sys.path.insert(0, "/opt/trn_rl_repo")

import numpy as np
from ml_dtypes import bfloat16

import concourse.bacc as bacc
import concourse.bass as bass
import concourse.mybir as mybir
from concourse import tile
from concourse.bass_utils import run_bass_kernel_spmd

BF = mybir.dt.bfloat16
F32 = mybir.dt.float32
F32R = mybir.dt.float32r
I16 = mybir.dt.int16
AF = mybir.ActivationFunctionType
ALU = mybir.AluOpType

P = 128          # partitions
DM = 768         # d_model
HD = 64          # head dim
NHC = 3          # heads per core
NCC = DM // P    # contraction chunks (6)
SQT = 512        # q-block (matmul moving dim)
QKV = 3 * NHC * HD  # 576

# Schraudolph bf16 exp: bf16_bits = round(x * 128/ln2 + (16256 - C))
SCH_S = 128.0 / float(np.log(2.0))
SCH_C = 6.5
DVE_EXP_MOD = 2  # full chunk ki -> DVE when ki % MOD != 0


def build_program(S=2048, n_devices=8):
    NS = S // P      # seq chunks of 128
    NQ = S // SQT    # q blocks of 512
    KPQ = SQT // P   # k-chunks per q-block (4)

    nc = bacc.Bacc(
        "TRN2", target_bir_lowering=False, debug=False, num_devices=n_devices
    )
    XB = 512
    NXB = S // XB
    xt_d = nc.dram_tensor("xt", [NXB, P, NCC, XB], BF, kind="ExternalInput")
    wqkv_d = nc.dram_tensor("wqkv", [P, NCC, QKV], BF, kind="ExternalInput")
    wo01_d = nc.dram_tensor("wo01", [P, DM], BF, kind="ExternalInput")
    wo2_d = nc.dram_tensor("wo2", [P, DM], BF, kind="ExternalInput")
    cos_d = nc.dram_tensor("cos", [P, NS, HD], BF, kind="ExternalInput")
    sin_d = nc.dram_tensor("sin", [P, NS, HD], BF, kind="ExternalInput")
    trimask_d = nc.dram_tensor("trimask", [P, P], BF, kind="ExternalInput")
    id_d = nc.dram_tensor("ident", [P, P], BF, kind="ExternalInput")
    out_d = nc.dram_tensor("outp", [S, DM], F32, kind="ExternalOutput")
    denr_dram = nc.dram_tensor("den_scratch", [2, SQT], F32, kind="Internal")

    with tile.TileContext(nc) as tc:
        with (
            tc.tile_pool(name="const", bufs=1) as const,
            tc.tile_pool(name="resid", bufs=1) as resid,
        ):
            qk_sb = resid.tile([P, NS, 384], BF)   # rope out: q 0:192 | k 192:384
            tz = resid.tile([P, 6, S], BF)         # j: 0..2 q heads, 3..5 k heads
            v_sb = resid.tile([P, NS, NHC, P], BF)
            ao01 = resid.tile([P, S], BF)
            ao2 = resid.tile([P, S], BF)
            aot = resid.tile([P, SQT], BF)         # h1 staging
            den_sb = resid.tile([P, 2, SQT], F32)  # row 0 live
            denr_sb = resid.tile([P, 2, SQT], F32)  # row 0 live
            wo01_sb = const.tile([P, DM], BF)
            wo2_sb = const.tile([P, DM], BF)
            cos_sb = const.tile([P, NS, HD], BF)
            sin_sb = const.tile([P, NS, HD], BF)
            trimask_sb = const.tile([P, P], BF)
            id_sb = const.tile([P, P], BF)

            # one-time pad zeroing, spread across engines (runs under the
            # initial input DMAs)
            nc.vector.memset(tz[HD:P, 0:3, :], 0.0)           # q transp pads
            nc.vector.memset(ao2[HD:P, :], 0.0)               # outproj K pad
            nc.gpsimd.memset(v_sb[:, :, :, HD], 1.0)          # denominator ones
            nc.gpsimd.memset(v_sb[:, :, :, HD + 1 : P], 0.0)  # v pad cols
            nc.gpsimd.memset(tz[HD:P, 3:6, :], 0.0)           # k transp pads

            wqkv_sb = const.tile([P, NCC, QKV], BF)
            nc.sync.dma_start(wqkv_sb[:, 0:3, :], wqkv_d[:, 0:3, :])
            nc.scalar.dma_start(wqkv_sb[:, 3:NCC, :], wqkv_d[:, 3:NCC, :])
            xt_sb = const.tile([P, NCC, S], BF)
            for b in range(NXB):
                bsl = slice(b * XB, (b + 1) * XB)
                csl = slice(b * (NS // NXB), (b + 1) * (NS // NXB))
                nc.sync.dma_start(cos_sb[:, csl, :], cos_d[:, csl, :])
                nc.scalar.dma_start(sin_sb[:, csl, :], sin_d[:, csl, :])
                nc.sync.dma_start(xt_sb[:, 0:3, bsl], xt_d[b][:, 0:3, :])
                nc.scalar.dma_start(xt_sb[:, 3:NCC, bsl], xt_d[b][:, 3:NCC, :])
            nc.sync.dma_start(id_sb[:], id_d[:])
            nc.scalar.dma_start(trimask_sb[:], trimask_d[:])
            nc.scalar.dma_start(wo01_sb[:], wo01_d[:])
            nc.sync.dma_start(wo2_sb[:], wo2_d[:])

            # ---- phase 1: qkv projection + rope + v pack + transposes ----
            with (
                tc.tile_pool(name="p1ps", bufs=3, space="PSUM") as pp,
                tc.tile_pool(name="p2ps", bufs=2, space="PSUM") as p2,
                tc.tile_pool(name="p1t", bufs=3) as tp,
            ):

                def emit_transposes(s):
                    pt = p2.tile([P, 384], BF, tag="pt", name="pt")
                    for j in range(3):
                        nc.tensor.transpose(
                            pt[:, j * P : (j + 1) * P],
                            qk_sb[:, s, j * P : (j + 1) * P],
                            id_sb[:],
                        )
                    # lo partitions of pt chunks -> tz slots 0,2,4
                    # hi partitions -> slots 1,3,5 (see head/slot mapping)
                    pstep = tz.ap[0][0]
                    lo = bass.AP(
                        tz.tensor, tz.offset + s * P,
                        [[pstep, HD], [2 * S, 3], [1, P]],
                    )
                    hi = bass.AP(
                        tz.tensor, tz.offset + S + s * P,
                        [[pstep, HD], [2 * S, 3], [1, P]],
                    )
                    src3 = pt.rearrange("p (a b) -> p a b", b=P)
                    nc.scalar.copy(lo, src3[0:HD, :, :])
                    nc.vector.tensor_copy(hi, src3[HD:P, :, :])

                for s in range(NS):
                    pqkv = pp.tile([P, QKV], F32, tag="pqkv", name="pqkv")
                    xsl = xt_sb[:, :, s * P : (s + 1) * P]
                    for c in range(NCC):
                        st, sp = (c == 0), (c == NCC - 1)
                        nc.tensor.matmul(
                            pqkv[:, 0:512], xsl[:, c, :], wqkv_sb[:, c, 0:512],
                            start=st, stop=sp,
                        )
                        nc.tensor.matmul(
                            pqkv[:, 512:QKV], xsl[:, c, :], wqkv_sb[:, c, 512:QKV],
                            start=st, stop=sp,
                        )
                    if s >= 1:
                        emit_transposes(s - 1)
                    qk = pqkv[:, 0:384]
                    qk3 = qk.rearrange("p (b x) -> p b x", x=32)
                    cs = cos_sb[:, s, :]
                    sn = sin_sb[:, s, :]
                    c3b = bass.AP(cs.tensor, cs.offset, [cs.ap[0], [0, 6], [1, HD]])
                    s3a = bass.AP(sn.tensor, sn.offset, [sn.ap[0], [0, 6], [32, 2], [1, 16]])
                    s3b = bass.AP(sn.tensor, sn.offset + 16, [sn.ap[0], [0, 6], [32, 2], [1, 16]])
                    t = tp.tile([P, 384], F32, tag="ropet", name="t")
                    t3 = t.rearrange("p (b x) -> p b x", x=32)
                    nc.vector.tensor_tensor(
                        t3[:, :, 0:16], qk3[:, :, 16:32], s3a, ALU.mult
                    )
                    nc.vector.tensor_tensor(
                        t3[:, :, 16:32], qk3[:, :, 0:16], s3b, ALU.mult
                    )
                    t2 = tp.tile([P, 384], F32, tag="ropet2", name="t2")
                    nc.vector.tensor_tensor(t2[:], qk[:], c3b, ALU.mult)
                    nc.gpsimd.tensor_tensor(
                        qk_sb[:, s, 0:192], t2[:, 0:192], t[:, 0:192], ALU.add
                    )
                    nc.gpsimd.tensor_tensor(
                        qk_sb[:, s, 192:384], t2[:, 192:384], t[:, 192:384], ALU.add
                    )
                    nc.scalar.copy(
                        v_sb[:, s, :, 0:HD],
                        pqkv[:, 384:QKV].rearrange("p (h x) -> p h x", x=HD),
                    )
                emit_transposes(NS - 1)

            # ---- phase 2: attention + den + outproj ----
            with (
                tc.tile_pool(name="scps", bufs=6, space="PSUM") as scp,
                tc.tile_pool(name="aops", bufs=2, space="PSUM") as aop,
                tc.tile_pool(name="expp", bufs=10) as expp,
                tc.tile_pool(name="outb", bufs=3) as outb,
                tc.tile_pool(name="denbp", bufs=2) as denbp,
            ):
                den_jobs = []  # (h, qj, pa) pending den-broadcast + scale

                def flush_den(keep):
                    while len(den_jobs) > keep:
                        h, qj, pa = den_jobs.pop(0)
                        slot = (NHC * qj + h) % 2
                        qsl = slice(qj * SQT, (qj + 1) * SQT)
                        denb = denbp.tile([P, SQT], F32, tag="denb", name="denb")
                        dr = denr_dram[slot, :]
                        bsrc = bass.AP(
                            dr.tensor, dr.offset, [[0, P], [1, SQT]]
                        )
                        nc.gpsimd.dma_start(denb[:], bsrc)
                        if h == 0:
                            dst = ao01[0:HD, qsl]
                        elif h == 1:
                            dst = aot[0:HD, :]
                        else:
                            dst = ao2[0:HD, qsl]
                        nc.vector.tensor_tensor(
                            dst, pa[0:HD, :], denb[0:HD, :], ALU.mult
                        )
                        if h == 1:
                            nc.scalar.copy(ao01[HD:P, qsl], aot[0:HD, :])

                def attention_head(qj, h):
                    nki = KPQ * qj + KPQ
                    flush_den(1)
                    pa = aop.tile([P, SQT], F32, tag="pa", name="pa")
                    av_q = []  # (ki, off, e, j)

                    def emit_av(lag):
                        while av_q and len(av_q) > lag:
                            ki, off, e, j = av_q.pop(0)
                            nc.tensor.matmul(
                                pa[:, off:SQT],
                                v_sb[:, ki, h, :],
                                e[:, off:SQT],
                                start=(ki == 0), stop=(ki == nki - 1),
                                skip_group_check=True,
                            )

                    for ki in range(nki):
                        r = ki - KPQ * qj
                        off = max(r, 0) * P
                        ps = scp.tile([P, SQT], F32, tag="ps", name="ps")
                        nc.tensor.matmul(
                            ps[:, off:SQT],
                            tz[:, 3 + h, ki * P : (ki + 1) * P],
                            tz[:, h, qj * SQT + off : (qj + 1) * SQT],
                            start=True, stop=(r < 0),
                            skip_group_check=True,
                        )
                        if r >= 0:
                            nc.tensor.matmul(
                                ps[:, off : off + P],
                                id_sb[:],
                                trimask_sb[:],
                                start=False, stop=True,
                                skip_group_check=True,
                            )
                        e = expp.tile([P, SQT], BF, tag="e", name="e")
                        if r < 0 and (ki % 2) != 0:
                            nc.vector.tensor_scalar(
                                e.bitcast(I16)[:, off:SQT],
                                ps[:, off:SQT],
                                SCH_S * 0.125,
                                16256.0 - SCH_C,
                                ALU.mult,
                                ALU.add,
                            )
                        else:
                            nc.scalar.activation(
                                e[:, off:SQT], ps[:, off:SQT], AF.Exp, scale=0.125
                            )
                        av_q.append((ki, off, e, 0))
                        emit_av(4)
                    emit_av(0)
                    slot = (NHC * qj + h) % 2
                    deng = nc.vector.tensor_copy if slot else nc.scalar.copy
                    deng(den_sb[0:1, slot, :], pa[HD : HD + 1, :])
                    nc.vector.reciprocal_approx_fast(
                        denr_sb[0:1, slot, :], den_sb[0:1, slot, :]
                    )
                    nc.gpsimd.dma_start(
                        denr_dram[slot, :], denr_sb[0:1, slot, :]
                    )
                    den_jobs.append((h, qj, pa))

                def outproj_block(qj):
                    for s in range(qj * KPQ, (qj + 1) * KPQ):
                        sl = slice(s * P, (s + 1) * P)
                        poa = scp.tile([P, SQT], F32, tag="ps", name="poa")
                        pob = scp.tile([P, SQT], F32, tag="ps", name="pob")
                        for po, lo, hi in ((poa, 0, 512), (pob, 512, DM)):
                            w = hi - lo
                            nc.tensor.matmul(
                                po[:, 0:w], ao01[:, sl], wo01_sb[:, lo:hi],
                                start=True, stop=False,
                            )
                            nc.tensor.matmul(
                                po[:, 0:w], ao2[:, sl], wo2_sb[:, lo:hi],
                                start=False, stop=True,
                            )
                        ob = outb.tile([P, DM], F32, tag="ob", name="ob")
                        nc.scalar.copy(ob[:, 0:512], poa[:, 0:512])
                        nc.vector.tensor_copy(ob[:, 512:DM], pob[:, 0:256])
                        nc.sync.dma_start(out_d[sl, :], ob[:])

                order = list(range(NQ - 1, -1, -1))
                for idx, qj in enumerate(order):
                    for h in range(NHC):
                        attention_head(qj, h)
                        if idx >= 1 and h == 1:
                            outproj_block(order[idx - 1])
                flush_den(0)
                outproj_block(order[-1])

    nc.compile()
    return nc


_cache = {}
LAST_RESULT = None


def _get_program(S, n_devices):
    key = (S, n_devices)
    if key not in _cache:
        _cache[key] = build_program(S, n_devices)
    return _cache[key]


def _rope_tables(row_ids, col_ids, S):
    inv = 1.0 / (10000.0 ** (np.arange(0, 32, 2, dtype=np.float64) / 32.0))

    def block(ids):
        ang = ids.astype(np.float64)[:, None] * inv[None, :]
        c = np.concatenate([np.cos(ang), np.cos(ang)], -1)
        s_ = np.concatenate([-np.sin(ang), np.sin(ang)], -1)  # signed (shuffle form)
        return c, s_

    cr, sr = block(np.asarray(row_ids))
    cc, sc = block(np.asarray(col_ids))
    cos64 = np.concatenate([cr, cc], -1)
    sin64 = np.concatenate([sr, sc], -1)
    return cos64.astype(bfloat16), sin64.astype(bfloat16)


def kernel(x, row_ids, col_ids, Wq, Wk, Wv, Wo):
    x = np.asarray(x)
    B, S, _ = x.shape
    n_cores = 8
    groups = n_cores // B  # head groups per batch (4)
    hpg = NHC

    nc = _get_program(S, n_cores)
    cos_t, sin_t = _rope_tables(row_ids, col_ids, S)
    cos_t = np.ascontiguousarray(cos_t.reshape(S // P, P, -1).transpose(1, 0, 2))
    sin_t = np.ascontiguousarray(sin_t.reshape(S // P, P, -1).transpose(1, 0, 2))

    pp_ = np.arange(P)[:, None]
    ff = np.arange(P)[None, :]
    trimask = np.where(pp_ > ff, -30000.0, 0.0).astype(np.float32).astype(bfloat16)
    ident = np.eye(P, dtype=bfloat16)

    Wq, Wk, Wv, Wo = (np.asarray(w, np.float32) for w in (Wq, Wk, Wv, Wo))
    in_maps = []
    for c in range(n_cores):
        b = c // groups
        h0 = (c % groups) * hpg
        rows = slice(h0 * HD, (h0 + hpg) * HD)
        xt = np.ascontiguousarray(x[b].T).astype(bfloat16)
        NXB = S // 512
        xt = np.ascontiguousarray(
            xt.reshape(NCC, P, NXB, 512).transpose(2, 1, 0, 3)
        )
        wqkv = np.concatenate(
            [Wq[rows].T, Wk[rows].T, Wv[rows].T], axis=1
        ).astype(bfloat16)
        wqkv = np.ascontiguousarray(wqkv.reshape(NCC, P, QKV).transpose(1, 0, 2))
        wo = np.ascontiguousarray(Wo[:, rows].T)  # [192, 768]
        wo01 = np.ascontiguousarray(wo[0:P, :]).astype(bfloat16)
        wo2 = np.zeros((P, DM), np.float32)
        wo2[0:HD, :] = wo[P : P + HD, :]
        wo2 = wo2.astype(bfloat16)
        in_maps.append(
            {
                "xt": xt,
                "wqkv": wqkv,
                "wo01": wo01,
                "wo2": wo2,
                "cos": cos_t,
                "sin": sin_t,
                "trimask": trimask,
                "ident": ident,
            }
        )

    import os

    trace = bool(os.environ.get("KERNEL_TRACE"))
    kw = {}
    if trace and os.environ.get("KERNEL_TRACE_DIR"):
        kw["tmpdir"] = os.environ["KERNEL_TRACE_DIR"]
    res = run_bass_kernel_spmd(nc, in_maps, list(range(n_cores)), trace=trace, **kw)
    global LAST_RESULT
    LAST_RESULT = res

    outs = [res.results[c]["outp"] for c in range(n_cores)]
    out = np.stack(
        [sum(outs[b * groups + g] for g in range(groups)) for b in range(B)], axis=0
    )
    return out.astype(np.float32)


# revision 22
# speedup vs baseline: 1.0265x; 1.0203x over previous
"""Trainium2 Bass kernel for causal self-attention with 2D RoPE.

Sharding: batch x head-group parallel over 8 NeuronCores.
  core c -> batch b = c // 4, heads h0 = (c % 4) * 3 .. h0+2.
Each core computes q/k/v projections for its 3 heads, 2D RoPE, causal
flash-attention, and a head-packed output projection accumulated in PSUM.
The host sums the 4 partial outputs per batch.

v2 design:
 - exp split between ACT (exact, scalar engine) and DVE (Schraudolph
   bf16 bit-trick; off-diagonal chunks only) to break the ACT exp wall.
 - causal diag mask added into score PSUM by an identity-stationary
   matmul of a -30000 tile (no vector ops, no memsets per tile).
 - denominator from an appended ones-column in V; per-q reciprocal via
   DVE approx-reciprocal on the [1,S] row; broadcast across partitions
   with a ones-row stationary matmul; attention output scaled by mult.
 - output projection: heads packed (h0,h1 | h2+pad) into K=128
   contractions accumulated in PSUM, DMA'd from PSUM straight to DRAM.
 - all matmuls K=128 so the PE stays in a single tile mode (no drains).
"""

import sys

sys.path.insert(0, "/opt/trn_rl_repo")

import numpy as np
from ml_dtypes import bfloat16

import concourse.bacc as bacc
import concourse.bass as bass
import concourse.mybir as mybir
from concourse import tile
from concourse.bass_utils import run_bass_kernel_spmd

BF = mybir.dt.bfloat16
F32 = mybir.dt.float32
F32R = mybir.dt.float32r
I16 = mybir.dt.int16
AF = mybir.ActivationFunctionType
ALU = mybir.AluOpType

P = 128          # partitions
DM = 768         # d_model
HD = 64          # head dim
NHC = 3          # heads per core
NCC = DM // P    # contraction chunks (6)
SQT = 512        # q-block (matmul moving dim)
QKV = 3 * NHC * HD  # 576

# Schraudolph bf16 exp: bf16_bits = round(x * 128/ln2 + (16256 - C))
SCH_S = 128.0 / float(np.log(2.0))
SCH_C = 6.5
DVE_EXP_MOD = 2  # full chunk ki -> DVE when ki % MOD != 0


def build_program(S=2048, n_devices=8):
    NS = S // P      # seq chunks of 128
    NQ = S // SQT    # q blocks of 512
    KPQ = SQT // P   # k-chunks per q-block (4)

    nc = bacc.Bacc(
        "TRN2", target_bir_lowering=False, debug=False, num_devices=n_devices
    )
    XB = 512
    NXB = S // XB
    xt_d = nc.dram_tensor("xt", [NXB, P, NCC, XB], BF, kind="ExternalInput")
    wqkv_d = nc.dram_tensor("wqkv", [P, NCC, QKV], BF, kind="ExternalInput")
    wo01_d = nc.dram_tensor("wo01", [P, DM], BF, kind="ExternalInput")
    wo2_d = nc.dram_tensor("wo2", [P, DM], BF, kind="ExternalInput")
    cos_d = nc.dram_tensor("cos", [P, NS, HD], BF, kind="ExternalInput")
    sin_d = nc.dram_tensor("sin", [P, NS, HD], BF, kind="ExternalInput")
    trimask_d = nc.dram_tensor("trimask", [P, P], BF, kind="ExternalInput")
    id_d = nc.dram_tensor("ident", [P, P], BF, kind="ExternalInput")
    out_d = nc.dram_tensor("outp", [S, DM], F32, kind="ExternalOutput")
    denr_dram = nc.dram_tensor("den_scratch", [2, SQT], F32, kind="Internal")

    with tile.TileContext(nc) as tc:
        with (
            tc.tile_pool(name="const", bufs=1) as const,
            tc.tile_pool(name="resid", bufs=1) as resid,
        ):
            qk_sb = resid.tile([P, NS, 384], BF)   # rope out: q 0:192 | k 192:384
            tz = resid.tile([P, 6, S], BF)         # j: 0..2 q heads, 3..5 k heads
            v_sb = resid.tile([P, NS, NHC, P], BF)
            ao01 = resid.tile([P, S], BF)
            ao2 = resid.tile([P, S], BF)
            aot = resid.tile([P, SQT], BF)         # h1 staging
            den_sb = resid.tile([P, 2, SQT], F32)  # row 0 live
            denr_sb = resid.tile([P, 2, SQT], F32)  # row 0 live
            wo01_sb = const.tile([P, DM], BF)
            wo2_sb = const.tile([P, DM], BF)
            cos_sb = const.tile([P, NS, HD], BF)
            sin_sb = const.tile([P, NS, HD], BF)
            trimask_sb = const.tile([P, P], BF)
            id_sb = const.tile([P, P], BF)

            # one-time pad zeroing, spread across engines (runs under the
            # initial input DMAs)
            nc.vector.memset(tz[HD:P, 0:3, :], 0.0)           # q transp pads
            nc.vector.memset(ao2[HD:P, :], 0.0)               # outproj K pad
            nc.gpsimd.memset(v_sb[:, :, :, HD], 1.0)          # denominator ones
            nc.gpsimd.memset(v_sb[:, :, :, HD + 1 : P], 0.0)  # v pad cols
            nc.gpsimd.memset(tz[HD:P, 3:6, :], 0.0)           # k transp pads

            wqkv_sb = const.tile([P, NCC, QKV], BF)
            nc.sync.dma_start(wqkv_sb[:, 0:3, :], wqkv_d[:, 0:3, :])
            nc.scalar.dma_start(wqkv_sb[:, 3:NCC, :], wqkv_d[:, 3:NCC, :])
            xt_sb = const.tile([P, NCC, S], BF)
            for b in range(NXB):
                bsl = slice(b * XB, (b + 1) * XB)
                csl = slice(b * (NS // NXB), (b + 1) * (NS // NXB))
                nc.sync.dma_start(cos_sb[:, csl, :], cos_d[:, csl, :])
                nc.scalar.dma_start(sin_sb[:, csl, :], sin_d[:, csl, :])
                nc.sync.dma_start(xt_sb[:, 0:3, bsl], xt_d[b][:, 0:3, :])
                nc.scalar.dma_start(xt_sb[:, 3:NCC, bsl], xt_d[b][:, 3:NCC, :])
            nc.sync.dma_start(id_sb[:], id_d[:])
            nc.scalar.dma_start(trimask_sb[:], trimask_d[:])
            nc.scalar.dma_start(wo01_sb[:], wo01_d[:])
            nc.sync.dma_start(wo2_sb[:], wo2_d[:])

            # ---- phase 1: qkv projection + rope + v pack + transposes ----
            with (
                tc.tile_pool(name="p1ps", bufs=3, space="PSUM") as pp,
                tc.tile_pool(name="p2ps", bufs=2, space="PSUM") as p2,
                tc.tile_pool(name="p1t", bufs=3) as tp,
            ):

                def emit_transposes(s):
                    pt = p2.tile([P, 384], BF, tag="pt", name="pt")
                    for j in range(3):
                        nc.tensor.transpose(
                            pt[:, j * P : (j + 1) * P],
                            qk_sb[:, s, j * P : (j + 1) * P],
                            id_sb[:],
                        )
                    # lo partitions of pt chunks -> tz slots 0,2,4
                    # hi partitions -> slots 1,3,5 (see head/slot mapping)
                    pstep = tz.ap[0][0]
                    lo = bass.AP(
                        tz.tensor, tz.offset + s * P,
                        [[pstep, HD], [2 * S, 3], [1, P]],
                    )
                    hi = bass.AP(
                        tz.tensor, tz.offset + S + s * P,
                        [[pstep, HD], [2 * S, 3], [1, P]],
                    )
                    src3 = pt.rearrange("p (a b) -> p a b", b=P)
                    nc.scalar.copy(lo, src3[0:HD, :, :])
                    nc.vector.tensor_copy(hi, src3[HD:P, :, :])

                for s in range(NS):
                    pqkv = pp.tile([P, QKV], F32, tag="pqkv", name="pqkv")
                    xsl = xt_sb[:, :, s * P : (s + 1) * P]
                    for c in range(NCC):
                        st, sp = (c == 0), (c == NCC - 1)
                        nc.tensor.matmul(
                            pqkv[:, 0:512], xsl[:, c, :], wqkv_sb[:, c, 0:512],
                            start=st, stop=sp,
                        )
                        nc.tensor.matmul(
                            pqkv[:, 512:QKV], xsl[:, c, :], wqkv_sb[:, c, 512:QKV],
                            start=st, stop=sp,
                        )
                    if s >= 1:
                        emit_transposes(s - 1)
                    qk = pqkv[:, 0:384]
                    qk3 = qk.rearrange("p (b x) -> p b x", x=32)
                    cs = cos_sb[:, s, :]
                    sn = sin_sb[:, s, :]
                    c3b = bass.AP(cs.tensor, cs.offset, [cs.ap[0], [0, 6], [1, HD]])
                    s3a = bass.AP(sn.tensor, sn.offset, [sn.ap[0], [0, 6], [32, 2], [1, 16]])
                    s3b = bass.AP(sn.tensor, sn.offset + 16, [sn.ap[0], [0, 6], [32, 2], [1, 16]])
                    t = tp.tile([P, 384], F32, tag="ropet", name="t")
                    t3 = t.rearrange("p (b x) -> p b x", x=32)
                    nc.vector.tensor_tensor(
                        t3[:, :, 0:16], qk3[:, :, 16:32], s3a, ALU.mult
                    )
                    nc.vector.tensor_tensor(
                        t3[:, :, 16:32], qk3[:, :, 0:16], s3b, ALU.mult
                    )
                    t2 = tp.tile([P, 384], F32, tag="ropet2", name="t2")
                    nc.vector.tensor_tensor(t2[:], qk[:], c3b, ALU.mult)
                    nc.gpsimd.tensor_tensor(
                        qk_sb[:, s, 0:192], t2[:, 0:192], t[:, 0:192], ALU.add
                    )
                    nc.gpsimd.tensor_tensor(
                        qk_sb[:, s, 192:384], t2[:, 192:384], t[:, 192:384], ALU.add
                    )
                    nc.scalar.copy(
                        v_sb[:, s, :, 0:HD],
                        pqkv[:, 384:QKV].rearrange("p (h x) -> p h x", x=HD),
                    )
                emit_transposes(NS - 1)

            # ---- phase 2: attention + den + outproj ----
            with (
                tc.tile_pool(name="scps", bufs=6, space="PSUM") as scp,
                tc.tile_pool(name="aops", bufs=2, space="PSUM") as aop,
                tc.tile_pool(name="expp", bufs=10) as expp,
                tc.tile_pool(name="outb", bufs=3) as outb,
                tc.tile_pool(name="denbp", bufs=2) as denbp,
            ):
                den_jobs = []  # (h, qj, pa) pending den-broadcast + scale

                def flush_den(keep):
                    while len(den_jobs) > keep:
                        h, qj, pa = den_jobs.pop(0)
                        slot = (NHC * qj + h) % 2
                        qsl = slice(qj * SQT, (qj + 1) * SQT)
                        denb = denbp.tile([P, SQT], F32, tag="denb", name="denb")
                        dr = denr_dram[slot, :]
                        bsrc = bass.AP(
                            dr.tensor, dr.offset, [[0, P], [1, SQT]]
                        )
                        nc.gpsimd.dma_start(denb[:], bsrc)
                        if h == 0:
                            dst = ao01[0:HD, qsl]
                        elif h == 1:
                            dst = aot[0:HD, :]
                        else:
                            dst = ao2[0:HD, qsl]
                        nc.vector.tensor_tensor(
                            dst, pa[0:HD, :], denb[0:HD, :], ALU.mult
                        )
                        if h == 1:
                            nc.scalar.copy(ao01[HD:P, qsl], aot[0:HD, :])

                def attention_head(qj, h):
                    nki = KPQ * qj + KPQ
                    flush_den(1)
                    pa = aop.tile([P, SQT], F32, tag="pa", name="pa")
                    av_q = []  # (ki, off, e, j)

                    def emit_av(lag):
                        while av_q and len(av_q) > lag:
                            ki, off, e, j = av_q.pop(0)
                            nc.tensor.matmul(
                                pa[:, off:SQT],
                                v_sb[:, ki, h, :],
                                e[:, off:SQT],
                                start=(ki == 0), stop=(ki == nki - 1),
                                skip_group_check=True,
                            )

                    for ki in range(nki):
                        r = ki - KPQ * qj
                        off = max(r, 0) * P
                        ps = scp.tile([P, SQT], F32, tag="ps", name="ps")
                        nc.tensor.matmul(
                            ps[:, off:SQT],
                            tz[:, 3 + h, ki * P : (ki + 1) * P],
                            tz[:, h, qj * SQT + off : (qj + 1) * SQT],
                            start=True, stop=(r < 0),
                            skip_group_check=True,
                        )
                        if r >= 0:
                            nc.tensor.matmul(
                                ps[:, off : off + P],
                                id_sb[:],
                                trimask_sb[:],
                                start=False, stop=True,
                                skip_group_check=True,
                            )
                        e = expp.tile([P, SQT], BF, tag="e", name="e")
                        if r < 0 and (ki % 2) != 0:
                            nc.vector.tensor_scalar(
                                e.bitcast(I16)[:, off:SQT],
                                ps[:, off:SQT],
                                SCH_S * 0.125,
                                16256.0 - SCH_C,
                                ALU.mult,
                                ALU.add,
                            )
                        else:
                            nc.scalar.activation(
                                e[:, off:SQT], ps[:, off:SQT], AF.Exp, scale=0.125
                            )
                        av_q.append((ki, off, e, 0))
                        emit_av(4)
                    emit_av(0)
                    slot = (NHC * qj + h) % 2
                    deng = nc.vector.tensor_copy if slot else nc.scalar.copy
                    deng(den_sb[0:1, slot, :], pa[HD : HD + 1, :])
                    nc.vector.reciprocal_approx_fast(
                        denr_sb[0:1, slot, :], den_sb[0:1, slot, :]
                    )
                    nc.gpsimd.dma_start(
                        denr_dram[slot, :], denr_sb[0:1, slot, :]
                    )
                    den_jobs.append((h, qj, pa))

                def outproj_block(qj):
                    for s in range(qj * KPQ, (qj + 1) * KPQ):
                        sl = slice(s * P, (s + 1) * P)
                        poa = scp.tile([P, SQT], F32, tag="ps", name="poa")
                        pob = scp.tile([P, SQT], F32, tag="ps", name="pob")
                        for po, lo, hi in ((poa, 0, 512), (pob, 512, DM)):
                            w = hi - lo
                            nc.tensor.matmul(
                                po[:, 0:w], ao01[:, sl], wo01_sb[:, lo:hi],
                                start=True, stop=False,
                            )
                            nc.tensor.matmul(
                                po[:, 0:w], ao2[:, sl], wo2_sb[:, lo:hi],
                                start=False, stop=True,
                            )
                        ob = outb.tile([P, DM], F32, tag="ob", name="ob")
                        nc.scalar.copy(ob[:, 0:512], poa[:, 0:512])
                        nc.vector.tensor_copy(ob[:, 512:DM], pob[:, 0:256])
                        nc.sync.dma_start(out_d[sl, :], ob[:])

                order = list(range(NQ - 1, -1, -1))
                for idx, qj in enumerate(order):
                    for h in range(NHC):
                        attention_head(qj, h)
                        if idx >= 1 and h == 1:
                            outproj_block(order[idx - 1])
                flush_den(0)
                outproj_block(order[-1])

    nc.compile()
    return nc


_cache = {}
LAST_RESULT = None


def _get_program(S, n_devices):
    key = (S, n_devices)
    if key not in _cache:
        _cache[key] = build_program(S, n_devices)
    return _cache[key]


def _rope_tables(row_ids, col_ids, S):
    inv = 1.0 / (10000.0 ** (np.arange(0, 32, 2, dtype=np.float64) / 32.0))

    def block(ids):
        ang = ids.astype(np.float64)[:, None] * inv[None, :]
        c = np.concatenate([np.cos(ang), np.cos(ang)], -1)
        s_ = np.concatenate([-np.sin(ang), np.sin(ang)], -1)  # signed (shuffle form)
        return c, s_

    cr, sr = block(np.asarray(row_ids))
    cc, sc = block(np.asarray(col_ids))
    cos64 = np.concatenate([cr, cc], -1)
    sin64 = np.concatenate([sr, sc], -1)
    return cos64.astype(bfloat16), sin64.astype(bfloat16)


def kernel(x, row_ids, col_ids, Wq, Wk, Wv, Wo):
    x = np.asarray(x)
    B, S, _ = x.shape
    n_cores = 8
    groups = n_cores // B  # head groups per batch (4)
    hpg = NHC

    nc = _get_program(S, n_cores)
    cos_t, sin_t = _rope_tables(row_ids, col_ids, S)
    cos_t = np.ascontiguousarray(cos_t.reshape(S // P, P, -1).transpose(1, 0, 2))
    sin_t = np.ascontiguousarray(sin_t.reshape(S // P, P, -1).transpose(1, 0, 2))

    pp_ = np.arange(P)[:, None]
    ff = np.arange(P)[None, :]
    trimask = np.where(pp_ > ff, -30000.0, 0.0).astype(np.float32).astype(bfloat16)
    ident = np.eye(P, dtype=bfloat16)

    Wq, Wk, Wv, Wo = (np.asarray(w, np.float32) for w in (Wq, Wk, Wv, Wo))
    in_maps = []
    for c in range(n_cores):
        b = c // groups
        h0 = (c % groups) * hpg
        rows = slice(h0 * HD, (h0 + hpg) * HD)
        xt = np.ascontiguousarray(x[b].T).astype(bfloat16)
        NXB = S // 512
        xt = np.ascontiguousarray(
            xt.reshape(NCC, P, NXB, 512).transpose(2, 1, 0, 3)
        )
        wqkv = np.concatenate(
            [Wq[rows].T, Wk[rows].T, Wv[rows].T], axis=1
        ).astype(bfloat16)
        wqkv = np.ascontiguousarray(wqkv.reshape(NCC, P, QKV).transpose(1, 0, 2))
        wo = np.ascontiguousarray(Wo[:, rows].T)  # [192, 768]
        wo01 = np.ascontiguousarray(wo[0:P, :]).astype(bfloat16)
        wo2 = np.zeros((P, DM), np.float32)
        wo2[0:HD, :] = wo[P : P + HD, :]
        wo2 = wo2.astype(bfloat16)
        in_maps.append(
            {
                "xt": xt,
                "wqkv": wqkv,
                "wo01": wo01,
                "wo2": wo2,
                "cos": cos_t,
                "sin": sin_t,
                "trimask": trimask,
                "ident": ident,
            }
        )

    import os

    trace = bool(os.environ.get("KERNEL_TRACE"))
    kw = {}
    if trace and os.environ.get("KERNEL_TRACE_DIR"):
        kw["tmpdir"] = os.environ["KERNEL_TRACE_DIR"]
    res = run_bass_kernel_spmd(nc, in_maps, list(range(n_cores)), trace=trace, **kw)
    global LAST_RESULT
    LAST_RESULT = res

    outs = [res.results[c]["outp"] for c in range(n_cores)]
    out = np.stack(
        [sum(outs[b * groups + g] for g in range(groups)) for b in range(B)], axis=0
    )
    return out.astype(np.float32)
